# revision 10
# baseline (speedup 1.0000x reference)
"""Trainium2 Bass kernel for nn_Block3D (dense_transformer).

Full inputs in, full outputs out. Internally shards H across 8 NeuronCores.

Layout per core: x slice [192, V] with V = T*HL*W = 16*8*64 = 8192 local
voxels (v = t*512 + h*64 + w), channels on SBUF partitions in two 96-row
chunks (chunk A = channels 0:96 = x_no, chunk B = 96:192 = x_yes).

Key algebraic restructurings (all host-side weight transforms):
  - LN weight/bias folded into the consuming 1x1 convs.
  - LN mean subtraction folded into column-centered GEMM weights (each
    weight row sums to zero -> invariant to per-voxel constant shifts),
    so the kernel only applies the per-voxel 1/std scale.
  - mat-attention pools commute with the 1x1 convs: pool raw (scaled)
    activations first, then tiny block-diagonal GEMMs on pooled tensors.
    Pool mean divisors are folded into those GEMM weights.
  - H-global pool partials (A1, A3, B2) are AllReduce'd across cores.
"""
import numpy as np

import concourse.bass as bass
import concourse.bacc as bacc
import concourse.tile as tile
import concourse.mybir as mybir
from concourse import bass_utils

F32 = mybir.dt.float32
F32R = mybir.dt.float32r
AF = mybir.ActivationFunctionType
OP = mybir.AluOpType

N_CORES = 8
C, C2, C6, HID = 192, 96, 32, 768
T, H, W = 16, 64, 64
HL = H // N_CORES
V = T * HL * W            # 8192
NVT = V // 512            # 16
EPS = 1e-6
TCH = [4, 8, 16]          # time-chunk length per ma branch

# AR buffer layout (columns): A1 [0:16], A3 [16:1040], B2E [1040:1296].
# B3E + XW live after ARW in SP and stay local.
ARW = 1296
O_A1, O_A3, O_B21, O_B31, O_XW = 0, 16, 1040, 1296, 1328
SCRP_W = 1456
# SCRR (recon) column offsets
O_Y1, O_Y2, O_Y3, O_B1U, O_Z2, O_Z3 = 0, 16, 144, 1168, 3216, 3472
SCRR_W = 3504

_CACHE = {}


def _mm(nc, out, lhsT, rhs, start=True, stop=True):
    nc.tensor.matmul(out, lhsT.bitcast(F32R), rhs.bitcast(F32R),
                     start=start, stop=stop)


def _emit(nc, tc, d, out_d):
    with nc.allow_low_precision(reason="float32r rounding on matmul inputs"), \
         tc.tile_pool(name="wp", bufs=1) as wpool, \
         tc.tile_pool(name="big", bufs=1) as bigp, \
         tc.tile_pool(name="sm", bufs=2) as smp, \
         tc.tile_pool(name="ps_st", bufs=2, space="PSUM") as ps_st, \
         tc.tile_pool(name="dram", bufs=1, space="DRAM") as dramp:

        # ---- persistent weights/constants (packed tiles) ----------------
        WMAT = wpool.tile([C2, 864], F32R, tag="wmat")
        nc.sync.dma_start(WMAT[:], d["wmat"][:, :].bitcast(F32R))
        BV = wpool.tile([C2, 8], F32, tag="bvec")
        nc.sync.dma_start(BV[:], d["bvec"])
        W1TA = wpool.tile([C2, HID], F32R, tag="w1ta")
        nc.sync.dma_start(W1TA[:], d["w1ta"][:, :].bitcast(F32R))
        W1TB = wpool.tile([C2, HID], F32R, tag="w1tb")
        nc.sync.dma_start(W1TB[:], d["w1tb"][:, :].bitcast(F32R))
        W2TP = wpool.tile([128, 1152], F32R, tag="w2tp")
        nc.sync.dma_start(W2TP[:], d["w2tp"][:, :].bitcast(F32R))
        T1V = wpool.tile([128, 6], F32, tag="t1v")
        nc.sync.dma_start(T1V[:], d["t1v"])

        wa = [WMAT[:, 96 * j:96 * (j + 1)] for j in range(3)]
        wb = [WMAT[:, 96 * (3 + j):96 * (4 + j)] for j in range(3)]
        wpt = WMAT[:, 576:672]
        ones1 = WMAT[:, 672:768]
        ones2 = WMAT[:, 768:864]
        bs = [BV[:, j:j + 1] for j in range(3)]
        bpv = BV[:, 3:4]
        b2a = BV[:, 4:5]
        b2b = BV[:, 5:6]
        epsb = BV[:, 6:7]

        x_d = d["x"]

        X1 = bigp.tile([C2, V], F32R, tag="X1")
        Y1P = bigp.tile([C2, V], F32, tag="bigA")
        OUT1B = bigp.tile([C2, V], F32R, tag="OUT1B")

        # ---- phase 1: LN1 stats + scaled activations --------------------
        for vt in range(NVT):
            sl = slice(512 * vt, 512 * (vt + 1))
            nc.sync.dma_start(X1[:, sl], x_d[C2:C, sl].bitcast(F32R))
            sq = smp.tile([C2, 512], F32R, tag="sq")
            nc.gpsimd.tensor_mul(sq[:], X1[:, sl], X1[:, sl])
            u = ps_st.tile([C2, 512], F32, tag="st")
            m2 = ps_st.tile([C2, 512], F32, tag="st")
            _mm(nc, u[:], ones1, X1[:, sl])
            _mm(nc, m2[:], ones1, sq[:])
            uv = smp.tile([C2, 512], F32, tag="uv")
            nc.scalar.square(uv[:], u[:])
            nc.vector.tensor_sub(uv[:], m2[:], uv[:])
            nc.scalar.activation(uv[:], uv[:], AF.Sqrt, bias=epsb)
            r = smp.tile([C2, 512], F32, tag="r")
            nc.vector.reciprocal_approx_fast(r[:], uv[:])
            nc.vector.tensor_mul(Y1P[:, sl], X1[:, sl], r[:])

        # ---- phase 2: local pools + AllReduce ---------------------------
        # All pooled tensors are built "epoch-aligned": for epoch e (t in
        # [4e,4e+4)), column block e holds, per row group, that branch's
        # time-chunk data (ma1 chunk e / ma2 chunk e//2 / ma3 chunk 0).
        # This lets every pooled GEMM be one full K=96 block-diag matmul
        # at base partition 0 (PSUM partition offsets are illegal ISA).
        with tc.tile_pool(name="pool2", bufs=1) as p2:
            SP = p2.tile([C2, SCRP_W], F32R, tag="scrp")
            XW = SP[:, O_XW:O_XW + T * HL]
            nc.vector.tensor_reduce(
                out=XW.rearrange("p (t h) -> p t h", t=T),
                in_=Y1P[:, :].rearrange("p (t h w) -> p t h w", t=T, h=HL),
                op=OP.add, axis=mybir.AxisListType.X)
            nc.vector.tensor_reduce(
                out=SP[:, O_A1:O_A1 + T],
                in_=XW.rearrange("p (t h) -> p t h", t=T),
                op=OP.add, axis=mybir.AxisListType.X)
            nc.vector.tensor_reduce(
                out=SP[:, O_A3:O_A3 + T * W].rearrange("p (t w) -> p t w", t=T),
                in_=Y1P[:, :].rearrange("p (t h w) -> p t w h", t=T, h=HL),
                op=OP.add, axis=mybir.AxisListType.X)
            # B3E [96, 32]: per-epoch, per-row-group sums over (t-chunk, w)
            for e in range(4):
                o = O_B31 + 8 * e
                nc.vector.tensor_reduce(
                    out=SP[0:32, o:o + 8],
                    in_=XW[0:32, 32 * e:32 * (e + 1)].rearrange(
                        "p (tl h) -> p h tl", tl=4),
                    op=OP.add, axis=mybir.AxisListType.X)
                nc.vector.tensor_reduce(
                    out=SP[32:64, o:o + 8],
                    in_=XW[32:64, 64 * (e // 2):64 * (e // 2 + 1)].rearrange(
                        "p (tl h) -> p h tl", tl=8),
                    op=OP.add, axis=mybir.AxisListType.X)
                nc.vector.tensor_reduce(
                    out=SP[64:96, o:o + 8],
                    in_=XW[64:96, :].rearrange("p (t h) -> p h t", t=T),
                    op=OP.add, axis=mybir.AxisListType.X)
            # S4E [96, 2048]: epoch-aligned time-chunk sums of Y1P
            S4E = p2.tile([C2, 4 * 512], F32R, tag="s4e")
            S8M = p2.tile([C2, 1024], F32R, tag="s8m")
            S16M = p2.tile([C2, 512], F32R, tag="s16m")
            nc.vector.tensor_reduce(
                out=S4E[0:32, :].rearrange("p (b f) -> p b f", b=4),
                in_=Y1P[0:32, :].rearrange("p (b tl f) -> p b f tl",
                                           b=4, tl=4),
                op=OP.add, axis=mybir.AxisListType.X)
            nc.vector.tensor_reduce(
                out=S8M[32:64, :].rearrange("p (b f) -> p b f", b=2),
                in_=Y1P[32:64, :].rearrange("p (b tl f) -> p b f tl",
                                            b=2, tl=8),
                op=OP.add, axis=mybir.AxisListType.X)
            nc.vector.tensor_reduce(
                out=S16M[64:96, :],
                in_=Y1P[64:96, :].rearrange("p (tl f) -> p f tl", tl=16),
                op=OP.add, axis=mybir.AxisListType.X)
            s8v = S8M[32:64, :].rearrange("p (b f) -> p b f", b=2)
            nc.scalar.copy(
                S4E[32:64, :].rearrange("p (b r f) -> p b r f", b=2, r=2),
                s8v[:, :, None, :].broadcast_to((32, 2, 2, 512)))
            nc.scalar.copy(
                S4E[64:96, :].rearrange("p (r f) -> p r f", r=4),
                S16M[64:96, None, :].broadcast_to((32, 4, 512)))
            # B2E [96, 256]: per-epoch sums over (t-chunk, h-local)
            nc.vector.tensor_reduce(
                out=SP[0:32, O_B21:O_B21 + 256].rearrange(
                    "p (b w) -> p b w", b=4),
                in_=S4E[0:32, :].rearrange("p (b h w) -> p b w h",
                                           b=4, h=HL),
                op=OP.add, axis=mybir.AxisListType.X)
            for e in range(4):
                o = O_B21 + 64 * e
                nc.vector.tensor_reduce(
                    out=SP[32:64, o:o + 64],
                    in_=S8M[32:64, 512 * (e // 2):512 * (e // 2 + 1)].rearrange(
                        "p (h w) -> p w h", h=HL),
                    op=OP.add, axis=mybir.AxisListType.X)
                nc.vector.tensor_reduce(
                    out=SP[64:96, o:o + 64],
                    in_=S16M[64:96, :].rearrange("p (h w) -> p w h", h=HL),
                    op=OP.add, axis=mybir.AxisListType.X)

            cin = dramp.tile([C2, ARW], F32R, tag="cin")
            cout = dramp.tile([C2, ARW], F32R, tag="cout")
            nc.sync.dma_start(cin[:, :], SP[:, 0:ARW])
            nc.gpsimd.collective_compute(
                "AllReduce", OP.add,
                replica_groups=[list(range(N_CORES))],
                ins=[cin.opt()], outs=[cout.opt()])
            GP = p2.tile([C2, ARW], F32R, tag="GP")
            nc.sync.dma_start(GP[:], cout[:])

            # ---- phase 3: pooled GEMMs + reconstruction -----------------
            SR = p2.tile([C2, SCRR_W], F32, tag="scrr")
            with tc.tile_pool(name="ps3", bufs=2, space="PSUM") as ps3:
                psY1 = ps3.tile([C2, T], F32, tag="accs")
                _mm(nc, psY1[:], wa[0], GP[:, 0:16])
                nc.scalar.activation(SR[:, O_Y1:O_Y1 + T], psY1[:],
                                     AF.Identity, bias=bs[0])
                psY2 = ps3.tile([C2, T * HL], F32, tag="accs")
                _mm(nc, psY2[:], wa[1], XW)
                nc.scalar.activation(SR[:, O_Y2:O_Y2 + T * HL], psY2[:],
                                     AF.Identity, bias=bs[1])
                for half in range(2):
                    psY3 = ps3.tile([C2, 512], F32, tag="acc")
                    _mm(nc, psY3[:], wa[2],
                        GP[:, 16 + 512 * half:16 + 512 * (half + 1)])
                    nc.scalar.activation(
                        SR[:, O_Y3 + 512 * half:O_Y3 + 512 * (half + 1)],
                        psY3[:], AF.Identity, bias=bs[2])

                for e in range(4):
                    psB = ps3.tile([C2, 512], F32, tag="acc")
                    _mm(nc, psB[:], wb[0], S4E[:, 512 * e:512 * (e + 1)])
                    nc.scalar.activation(
                        SR[:, O_B1U + 512 * e:O_B1U + 512 * (e + 1)], psB[:],
                        AF.Identity, bias=bs[0])

                psZ2 = ps3.tile([C2, 256], F32, tag="accs")
                for e in range(4):
                    zsl = slice(64 * e, 64 * (e + 1))
                    _mm(nc, psZ2[:, zsl], wb[1],
                        GP[:, O_B21 + 64 * e:O_B21 + 64 * (e + 1)])
                nc.scalar.activation(SR[:, O_Z2:O_Z2 + 256], psZ2[:],
                                     AF.Identity, bias=bs[1])

                psZ3 = ps3.tile([C2, 32], F32, tag="accs")
                for e in range(4):
                    zsl = slice(8 * e, 8 * (e + 1))
                    _mm(nc, psZ3[:, zsl], wb[2],
                        SP[:, O_B31 + 8 * e:O_B31 + 8 * (e + 1)])
                nc.scalar.activation(SR[:, O_Z3:O_Z3 + 32], psZ3[:],
                                     AF.Identity, bias=bs[2])

                # Uall[c,t,w] = Y3*Z2u[b(t),w] (in place over Y3)
                Y3s = SR[:, O_Y3:O_Y3 + T * W]
                z2v = SR[:, O_Z2:O_Z2 + 256].rearrange("p (e w) -> p e w", e=4)
                nc.vector.tensor_mul(
                    Y3s, Y3s, z2v[:, :, None, :].broadcast_to((C2, 4, 4, W)))
                # tmp2[c,t,h] = Y2 * Z3u[b(t),h] * Y1[t] (in place over Y2)
                Y2s = SR[:, O_Y2:O_Y2 + T * HL]
                z3v = SR[:, O_Z3:O_Z3 + 32].rearrange("p (e h) -> p e h", e=4)
                nc.vector.tensor_mul(
                    Y2s, Y2s, z3v[:, :, None, :].broadcast_to((C2, 4, 4, HL)))
                nc.vector.tensor_mul(
                    Y2s, Y2s,
                    SR[:, O_Y1:O_Y1 + T][:, :, None].broadcast_to((C2, T, HL)))

                G = bigp.tile([C2, V], F32R, tag="bigA")
                b1v = SR[:, O_B1U:O_B1U + 2048].rearrange(
                    "p (e f) -> p e f", e=4)
                uav = Y3s.rearrange("p (t w) -> p t w", t=T)
                nc.vector.tensor_mul(
                    G[:, :],
                    b1v[:, :, None, :].broadcast_to((C2, 4, 4, 512)),
                    uav[:, :, None, :].broadcast_to((C2, T, HL, W)))
                t2v = Y2s.rearrange("p (t h) -> p t h", t=T)
                nc.vector.tensor_mul(
                    G[:, :], G[:, :],
                    t2v[:, :, :, None].broadcast_to((C2, T, HL, W)))

                # wp GEMM + residual -> OUT1B
                for vt in range(NVT):
                    sl = slice(512 * vt, 512 * (vt + 1))
                    psW = ps3.tile([C2, 512], F32, tag="acc")
                    _mm(nc, psW[:], wpt, G[:, sl])
                    nc.vector.scalar_tensor_tensor(
                        OUT1B[:, sl], psW[:], bpv, X1[:, sl],
                        op0=OP.add, op1=OP.add)

        # ---- phase 4: LN2 + MLP + residual ------------------------------
        X0 = bigp.tile([C2, V], F32R, tag="X1")
        with tc.tile_pool(name="ps4", bufs=3, space="PSUM") as ps4, \
             tc.tile_pool(name="ps4b", bufs=2, space="PSUM") as ps4b:
            for vt in range(NVT):
                sl = slice(512 * vt, 512 * (vt + 1))
                nc.sync.dma_start(X0[:, sl], x_d[0:C2, sl].bitcast(F32R))
                sqA = smp.tile([C2, 512], F32R, tag="sq")
                nc.gpsimd.tensor_mul(sqA[:], X0[:, sl], X0[:, sl])
                sqB = smp.tile([C2, 512], F32R, tag="sqB")
                nc.gpsimd.tensor_mul(sqB[:], OUT1B[:, sl], OUT1B[:, sl])
                u = ps_st.tile([C2, 512], F32, tag="st")
                m2 = ps_st.tile([C2, 512], F32, tag="st")
                _mm(nc, u[:], ones2, X0[:, sl], start=True, stop=False)
                _mm(nc, u[:], ones2, OUT1B[:, sl], start=False, stop=True)
                _mm(nc, m2[:], ones2, sqA[:], start=True, stop=False)
                _mm(nc, m2[:], ones2, sqB[:], start=False, stop=True)
                uv = smp.tile([C2, 512], F32, tag="uv")
                nc.scalar.square(uv[:], u[:])
                nc.vector.tensor_sub(uv[:], m2[:], uv[:])
                nc.scalar.activation(uv[:], uv[:], AF.Sqrt, bias=epsb)
                r = smp.tile([C2, 512], F32, tag="r")
                nc.vector.reciprocal_approx_fast(r[:], uv[:])
                yA = smp.tile([C2, 512], F32R, tag="yA")
                nc.vector.tensor_mul(yA[:], X0[:, sl], r[:])
                yB = smp.tile([C2, 512], F32R, tag="yB")
                nc.vector.tensor_mul(yB[:], OUT1B[:, sl], r[:])

                ps2a = ps4b.tile([C2, 512], F32, tag="f2")
                ps2b = ps4b.tile([C2, 512], F32, tag="f2")
                for hc in range(6):
                    csl = slice(128 * hc, 128 * (hc + 1))
                    psF = ps4.tile([128, 512], F32, tag="mm")
                    _mm(nc, psF[:], W1TA[:, csl], yA[:], start=True, stop=False)
                    _mm(nc, psF[:], W1TB[:, csl], yB[:], start=False, stop=True)
                    hh = smp.tile([128, 512], F32R, tag="hh")
                    nc.scalar.activation(hh[:], psF[:], AF.Gelu,
                                         bias=T1V[:, hc:hc + 1])
                    _mm(nc, ps2a[:], W2TP[:, 192 * hc:192 * hc + 96], hh[:],
                        start=(hc == 0), stop=(hc == 5))
                    _mm(nc, ps2b[:], W2TP[:, 192 * hc + 96:192 * (hc + 1)], hh[:],
                        start=(hc == 0), stop=(hc == 5))
                oA = smp.tile([C2, 512], F32, tag="oA")
                nc.vector.scalar_tensor_tensor(
                    oA[:], ps2a[:], b2a, X0[:, sl].bitcast(F32),
                    op0=OP.add, op1=OP.add)
                oB = smp.tile([C2, 512], F32, tag="oB")
                nc.vector.scalar_tensor_tensor(
                    oB[:], ps2b[:], b2b, OUT1B[:, sl].bitcast(F32),
                    op0=OP.add, op1=OP.add)
                nc.sync.dma_start(out_d[0:C2, sl], oA[:])
                nc.sync.dma_start(out_d[C2:C, sl], oB[:])


def _build():
    if "nc" in _CACHE:
        return _CACHE["nc"]
    nc = bacc.Bacc("TRN2", target_bir_lowering=False, debug=False,
                   num_devices=N_CORES)
    d = {}

    def din(name, shape, dt=F32):
        d[name] = nc.dram_tensor(name, list(shape), dt,
                                 kind="ExternalInput").ap()

    din("x", (C, V))
    din("wmat", (C2, 864))
    din("bvec", (C2, 8))
    din("w1ta", (C2, HID))
    din("w1tb", (C2, HID))
    din("w2tp", (128, 1152))
    din("t1v", (128, 6))
    out_d = nc.dram_tensor("out", [C, V], F32, kind="ExternalOutput").ap()

    with tile.TileContext(nc) as tc:
        _emit(nc, tc, d, out_d)
    nc.compile()
    _CACHE["nc"] = nc
    return nc


def _blockdiag(mats):
    out = np.zeros((C2, C2), np.float64)
    for m, w in enumerate(mats):
        out[32 * m:32 * (m + 1), 32 * m:32 * (m + 1)] = w
    return out


def _prep_weights(params):
    g = lambda a: np.asarray(a, np.float64)
    n1w, n1b = g(params["n1_w"]), g(params["n1_b"])
    n2w, n2b = g(params["n2_w"]), g(params["n2_b"])
    fc1w, fc1b = g(params["fc1_w"]), g(params["fc1_b"])
    fc2w, fc2b = g(params["fc2_w"]), g(params["fc2_b"])
    mas = [params["ma1"], params["ma2"], params["ma3"]]

    out = {}
    f32 = lambda a: np.ascontiguousarray(a, np.float32)

    # fc1 folded with LN2 weight, column-centered; bias folded with LN2 bias
    W1F = fc1w * n2w[None, :]
    W1C = W1F - W1F.mean(axis=1, keepdims=True)
    t1 = fc1b + fc1w @ n2b
    w1t = W1C.T  # [192, 768]
    out["w1ta"] = f32(w1t[:C2])
    out["w1tb"] = f32(w1t[C2:])
    w2t = fc2w.T  # [768, 192]
    w2tp = np.zeros((128, 1152), np.float64)
    t1v = np.zeros((128, 6), np.float64)
    for h in range(6):
        w2tp[:, 192 * h:192 * (h + 1)] = w2t[128 * h:128 * (h + 1)]
        t1v[:, h] = t1[128 * h:128 * (h + 1)]
    out["w2tp"] = f32(w2tp)
    out["t1v"] = f32(t1v)

    # ma weights: fold LN1 weight, center, fold pool divisors
    WC = [[None] * 3 for _ in range(3)]   # [branch m][conv j]
    BF = [[None] * 3 for _ in range(3)]
    for m, ma in enumerate(mas):
        slc = slice(32 * m, 32 * (m + 1))
        for j, (wk, bk) in enumerate((("w1", "b1"), ("w2", "b2"), ("w3", "b3"))):
            wj, bj = g(ma[wk]), g(ma[bk])
            WF = wj * n1w[slc][None, :]
            WC[m][j] = WF - WF.mean(axis=1, keepdims=True)
            BF[m][j] = bj + wj @ n1b[slc]
    wmat = np.zeros((C2, 864), np.float64)
    wmat[:, 0:96] = _blockdiag([WC[m][0].T / (H * W) for m in range(3)])
    wmat[:, 96:192] = _blockdiag([WC[m][1].T / W for m in range(3)])
    wmat[:, 192:288] = _blockdiag([WC[m][2].T / H for m in range(3)])
    wmat[:, 288:384] = _blockdiag([WC[m][0].T / TCH[m] for m in range(3)])
    wmat[:, 384:480] = _blockdiag([WC[m][1].T / (TCH[m] * H) for m in range(3)])
    wmat[:, 480:576] = _blockdiag([WC[m][2].T / (TCH[m] * W) for m in range(3)])
    wmat[:, 576:672] = _blockdiag([g(mas[m]["wp"]).T for m in range(3)])
    wmat[:, 672:768] = 1.0 / C2
    wmat[:, 768:864] = 1.0 / C
    out["wmat"] = f32(wmat)

    bvec = np.zeros((C2, 8), np.float64)
    for j in range(3):
        bvec[:, j] = np.concatenate([BF[m][j] for m in range(3)])
    bvec[:, 3] = np.concatenate([g(mas[m]["bp"]) for m in range(3)])
    bvec[:, 4] = fc2b[:C2]
    bvec[:, 5] = fc2b[C2:]
    bvec[:, 6] = EPS
    out["bvec"] = f32(bvec)
    return out


def kernel(x, params):
    x = np.asarray(x, np.float32)
    assert x.shape == (1, C, T, H, W)
    wts = _prep_weights(params)

    nc = _build()
    in_maps = []
    for k in range(N_CORES):
        xs = np.ascontiguousarray(
            x[0, :, :, HL * k:HL * (k + 1), :], np.float32).reshape(C, V)
        m = {"x": xs}
        m.update(wts)
        in_maps.append(m)

    res = bass_utils.run_bass_kernel_spmd(
        nc, in_maps, core_ids=list(range(N_CORES)))

    y = np.empty((1, C, T, H, W), np.float32)
    for k in range(N_CORES):
        y[0, :, :, HL * k:HL * (k + 1), :] = \
            res.results[k]["out"].reshape(C, T, HL, W)
    return y
